# revision 3
# baseline (speedup 1.0000x reference)
"""Trainium2 Bass kernel for a pre-norm transformer block (MHSA + FFN).

Sharding: 8 cores, data parallel over (batch, seq-half). Core c handles
batch c//2, sequence half c%2. Inputs are permuted so each core's own
1024 tokens come first; attention K/V run over all 2048 tokens of the
batch (softmax is permutation invariant).

Matmul dtypes: f32r (TF32-like, ~1.5e-4 rel err) everywhere except the
FFN second half (h1/W2 in bf16). Softmax uses a constant exp shift
(logits are ~N(0, 26^2); exp(l - 128) stays inside fp32 range) and the
denominator is computed by a ones-column folded into the PV matmul,
normalized during the small o-transpose.
"""
import contextlib

import numpy as np
import ml_dtypes

import concourse.bass as bass
import concourse.tile as tile
import concourse.mybir as mybir
from concourse.bass_utils import run_bass_kernel_spmd
from concourse.masks import make_identity

B, T, C = 4, 2048, 1024
H, DH = 16, 64
DFF = 4 * C
N_CORES = 8
TQ = T // 2          # tokens owned per core
TS = T               # key/value tokens per core
NKO = C // 128       # 8 contraction tiles for C
F32R = mybir.dt.float32r
F32 = mybir.dt.float32
BF16 = mybir.dt.bfloat16
EXP_BIAS = -128.0
EPS = 1e-5

# ---------------------------------------------------------------------------
# Compat: this walrus build accepts at most 1 sem-wait per regular
# instruction (2 per InstEventSemaphore). bacc misses some tile-generated
# instructions, so split waits ourselves after finalize.
_ev_counter = [0]


def _legalize_sem_waits(nc):
    for func in nc.m.functions:
        for bb in func.blocks:
            new = []
            changed = False
            for inst in bb.instructions:
                si = inst.sync_info
                cap = 2 if isinstance(inst, mybir.InstEventSemaphore) else 1
                if si is not None and len(si.on_wait) > cap:
                    waits = list(si.on_wait)
                    for i in range(cap, len(waits), 2):
                        _ev_counter[0] += 1
                        e = mybir.InstEventSemaphore(
                            name=f"EVSPLIT-{_ev_counter[0]}", ins=[], outs=[])
                        e.engine = inst.engine
                        e.sync_info = mybir.SyncInfo(
                            on_wait=waits[i:i + 2], on_update=[])
                        new.append(e)
                    inst.sync_info = mybir.SyncInfo(
                        on_wait=waits[:cap], on_update=list(si.on_update))
                    changed = True
                new.append(inst)
            if changed:
                bb.instructions = new


# ---------------------------------------------------------------------------

def _layernorm_tile(nc, stats, work, x_ap, g_rep, b_rep, eps_t, out_ap):
    """LN over the free dim (1024) of x_ap [128, 1024] -> out_ap (any dtype)."""
    st = stats.tile([128, 2, 6], F32, tag="bnstats")
    mv = stats.tile([128, 2], F32, tag="bnaggr")
    xg = x_ap.rearrange("p (s d) -> p s d", s=2)
    for s in range(2):
        nc.vector.bn_stats(out=st[:, s, :], in_=xg[:, s, :])
    nc.vector.bn_aggr(out=mv[:], in_=st[:])
    rstd = stats.tile([128, 1], F32, tag="rstd")
    nc.scalar.activation(out=rstd[:], in_=mv[:, 1:2],
                         func=mybir.ActivationFunctionType.Sqrt,
                         bias=eps_t[:], scale=1.0)
    nc.vector.reciprocal(out=rstd[:], in_=rstd[:])
    xn = work.tile([128, 1024], F32, tag="ln_xn")
    nc.vector.tensor_scalar(out=xn[:], in0=x_ap,
                            scalar1=mv[:, 0:1], scalar2=rstd[:],
                            op0=mybir.AluOpType.subtract,
                            op1=mybir.AluOpType.mult)
    nc.vector.tensor_mul(out=xn[:], in0=xn[:], in1=g_rep[:])
    nc.vector.tensor_add(out=out_ap, in0=xn[:], in1=b_rep[:])


def _build_nc():
    nc = bass.Bass()

    # ---- I/O ----
    x_d = nc.dram_tensor("x", [T, C], F32, kind="ExternalInput")
    wq_d = nc.dram_tensor("wq", [C, C], F32R, kind="ExternalInput")
    wk_d = nc.dram_tensor("wk", [C, C], F32R, kind="ExternalInput")
    wv_d = nc.dram_tensor("wv", [C, C], F32R, kind="ExternalInput")
    wo_d = nc.dram_tensor("wo", [C, C], F32R, kind="ExternalInput")
    w1_d = nc.dram_tensor("w1", [C, DFF], F32R, kind="ExternalInput")
    w2_d = nc.dram_tensor("w2", [DFF, C], BF16, kind="ExternalInput")
    bq_d = nc.dram_tensor("bq", [C], F32, kind="ExternalInput")
    bk_d = nc.dram_tensor("bk", [C], F32, kind="ExternalInput")
    bv_d = nc.dram_tensor("bv", [C], F32, kind="ExternalInput")
    bo_d = nc.dram_tensor("bo", [C], F32, kind="ExternalInput")
    b1_d = nc.dram_tensor("b1", [DFF], F32, kind="ExternalInput")
    b2_d = nc.dram_tensor("b2", [C], F32, kind="ExternalInput")
    ln1g_d = nc.dram_tensor("ln1g", [C], F32, kind="ExternalInput")
    ln1b_d = nc.dram_tensor("ln1b", [C], F32, kind="ExternalInput")
    ln2g_d = nc.dram_tensor("ln2g", [C], F32, kind="ExternalInput")
    ln2b_d = nc.dram_tensor("ln2b", [C], F32, kind="ExternalInput")
    out_d = nc.dram_tensor("out", [TQ, C], F32, kind="ExternalOutput")

    # ---- HBM scratch ----
    qT_h = nc.dram_tensor("qT_h", [NKO, 128, TQ], F32R)
    kT_h = nc.dram_tensor("kT_h", [NKO, 128, TS], F32R)
    v_h = nc.dram_tensor("v_h", [TS // 128, 128, C], F32R)
    x2_h = nc.dram_tensor("x2_h", [TQ // 128, 128, C], F32)

    def bcast(ap, p=128):
        return bass.AP(tensor=ap.tensor, offset=ap.offset,
                       ap=[[0, p]] + [list(x) for x in ap.ap])

    with tile.TileContext(nc) as tc:
        with contextlib.ExitStack() as top:
            consts = top.enter_context(tc.tile_pool(name="consts", bufs=1))
            arena = top.enter_context(tc.tile_pool(name="arena", bufs=1))
            stats = top.enter_context(tc.tile_pool(name="stats", bufs=4))
            ps = top.enter_context(tc.tile_pool(name="ps", bufs=6, space="PSUM"))
            pst = top.enter_context(tc.tile_pool(name="pst", bufs=2, space="PSUM"))

            ident_f = consts.tile([128, 128], F32, tag="identf")
            make_identity(nc, ident_f)
            ident_r = consts.tile([128, 128], F32R, tag="identr")
            nc.vector.tensor_copy(out=ident_r[:], in_=ident_f[:])
            ebias = consts.tile([128, 1], F32, tag="ebias")
            nc.vector.memset(ebias[:], EXP_BIAS)
            eps_t = consts.tile([128, 1], F32, tag="eps")
            nc.vector.memset(eps_t[:], EPS)
            bq_s = consts.tile([128, NKO], F32, tag="bq")
            bk_s = consts.tile([128, NKO], F32, tag="bk")
            bo_s = consts.tile([128, NKO], F32, tag="bo")
            b2_s = consts.tile([128, NKO], F32, tag="b2")
            b1_s = consts.tile([128, DFF // 128], F32, tag="b1")
            for dst, src in ((bq_s, bq_d), (bk_s, bk_d), (bo_s, bo_d), (b2_s, b2_d), (b1_s, b1_d)):
                nc.sync.dma_start(out=dst[:], in_=src.rearrange("(o p) -> p o", p=128))
            bv_r = consts.tile([128, C], F32, tag="bvr")
            nc.gpsimd.dma_start(out=bv_r[:], in_=bcast(bv_d[:]))

            # ------------- Stage A: LN1 + transpose -> xnT -------------
            xnT = arena.tile([128, NKO, T], F32R, tag="arena")
            with tc.tile_pool(name="lnA", bufs=1) as lnA, \
                 tc.tile_pool(name="workA", bufs=2) as workA:
                ln1g_r = lnA.tile([128, C], F32, tag="g")
                ln1b_r = lnA.tile([128, C], F32, tag="b")
                nc.gpsimd.dma_start(out=ln1g_r[:], in_=bcast(ln1g_d[:]))
                nc.gpsimd.dma_start(out=ln1b_r[:], in_=bcast(ln1b_d[:]))
                for t in range(T // 128):
                    x_t = workA.tile([128, C], F32, tag="x_t")
                    nc.sync.dma_start(out=x_t[:], in_=x_d[t * 128:(t + 1) * 128, :])
                    xn_r = workA.tile([128, C], F32R, tag="xn_r")
                    _layernorm_tile(nc, stats, workA, x_t[:], ln1g_r, ln1b_r, eps_t, xn_r[:])
                    for c in range(NKO):
                        pt = pst.tile([128, 128], F32R, tag="pst")
                        nc.tensor.transpose(pt[:], xn_r[:, c * 128:(c + 1) * 128],
                                            ident_r[:])
                        nc.scalar.copy(out=xnT[:, c, t * 128:(t + 1) * 128], in_=pt[:])

            # ------------- Stage B: QKV -------------
            with tc.tile_pool(name="wres", bufs=1) as wres, \
                 tc.tile_pool(name="stage", bufs=4) as stage:
                wq_s = wres.tile([128, NKO, C], F32R, tag="wq")
                wk_s = wres.tile([128, NKO, C], F32R, tag="wk")
                wv_s = wres.tile([128, NKO, C], F32R, tag="wv")
                nc.sync.dma_start(out=wq_s[:], in_=wq_d.rearrange("(o p) f -> p o f", p=128))
                nc.sync.dma_start(out=wk_s[:], in_=wk_d.rearrange("(o p) f -> p o f", p=128))
                nc.sync.dma_start(out=wv_s[:], in_=wv_d.rearrange("(o p) f -> p o f", p=128))

                for f in range(NKO):
                    for ch in range(TQ // 512):
                        pq = ps.tile([128, 512], F32, tag="ps")
                        for ko in range(NKO):
                            nc.tensor.matmul(pq[:], wq_s[:, ko, f * 128:(f + 1) * 128],
                                             xnT[:, ko, ch * 512:(ch + 1) * 512],
                                             start=(ko == 0), stop=(ko == NKO - 1))
                        st = stage.tile([128, 512], F32R, tag="stg")
                        nc.scalar.activation(out=st[:], in_=pq[:],
                                             func=mybir.ActivationFunctionType.Identity,
                                             bias=bq_s[:, f:f + 1], scale=1.0)
                        nc.sync.dma_start(out=qT_h[f, :, ch * 512:(ch + 1) * 512], in_=st[:])
                for f in range(NKO):
                    for ch in range(TS // 512):
                        pk = ps.tile([128, 512], F32, tag="ps")
                        for ko in range(NKO):
                            nc.tensor.matmul(pk[:], wk_s[:, ko, f * 128:(f + 1) * 128],
                                             xnT[:, ko, ch * 512:(ch + 1) * 512],
                                             start=(ko == 0), stop=(ko == NKO - 1))
                        st = stage.tile([128, 512], F32R, tag="stg")
                        nc.scalar.activation(out=st[:], in_=pk[:],
                                             func=mybir.ActivationFunctionType.Identity,
                                             bias=bk_s[:, f:f + 1], scale=1.0)
                        nc.sync.dma_start(out=kT_h[f, :, ch * 512:(ch + 1) * 512], in_=st[:])
                for to in range(TS // 128):
                    for ch in range(C // 512):
                        pv = ps.tile([128, 512], F32, tag="ps")
                        for ko in range(NKO):
                            nc.tensor.matmul(pv[:], xnT[:, ko, to * 128:(to + 1) * 128],
                                             wv_s[:, ko, ch * 512:(ch + 1) * 512],
                                             start=(ko == 0), stop=(ko == NKO - 1))
                        st = stage.tile([128, 512], F32R, tag="stg")
                        nc.vector.tensor_add(out=st[:], in0=pv[:],
                                             in1=bv_r[:, ch * 512:(ch + 1) * 512])
                        nc.sync.dma_start(out=v_h[to, :, ch * 512:(ch + 1) * 512], in_=st[:])

            with contextlib.ExitStack() as late:
                fm4 = late.enter_context(tc.tile_pool(name="fm4", bufs=1))

                # ------------- Stage C: attention -------------
                oT = fm4.tile([128, NKO, TQ], F32R, tag="fm4")
                with tc.tile_pool(name="attn", bufs=2) as at, \
                     tc.tile_pool(name="probs", bufs=1) as prb, \
                     tc.tile_pool(name="att_sm", bufs=4) as asm:
                    for pair in range(H // 2):
                        qp = at.tile([128, TQ], F32R, tag="qp")
                        kp = at.tile([128, TS], F32R, tag="kp")
                        nc.sync.dma_start(out=qp[:], in_=qT_h[pair])
                        nc.sync.dma_start(out=kp[:], in_=kT_h[pair])
                        opair = at.tile([128, TQ // 128, 128], F32, tag="opair")
                        for h2 in range(2):
                            h = pair * 2 + h2
                            base = h2 * 64
                            vt = at.tile([128, TS // 128, 72], F32R, tag="vt")
                            nc.sync.dma_start(
                                out=vt[:, :, 0:DH],
                                in_=v_h[:, :, h * DH:(h + 1) * DH].rearrange("o p d -> p o d"))
                            nc.vector.memset(vt[:, :, DH:DH + 1].bitcast(F32), 1.0)
                            for ch in range(TQ // 512):
                                probsT = prb.tile([128, TS // 128, 512], F32R, tag="probsT")
                                for tso in range(TS // 128):
                                    sT = ps.tile([128, 512], F32, tag="ps")
                                    nc.tensor.matmul(
                                        sT[:], kp[base:base + DH, tso * 128:(tso + 1) * 128],
                                        qp[base:base + DH, ch * 512:(ch + 1) * 512],
                                        start=True, stop=True)
                                    nc.scalar.activation(
                                        out=probsT[:, tso, :], in_=sT[:],
                                        func=mybir.ActivationFunctionType.Exp,
                                        scale=8.0, bias=ebias[:])
                                ov = ps.tile([128, 512], F32, tag="ps")
                                for to in range(TS // 128):
                                    nc.tensor.matmul(ov[0:DH + 1, :], vt[:, to, 0:DH + 1],
                                                     probsT[:, to, :],
                                                     start=(to == 0), stop=(to == TS // 128 - 1))
                                ouT = asm.tile([72, 512], F32R, tag="ouT")
                                nc.scalar.copy(out=ouT[0:DH + 1, :], in_=ov[0:DH + 1, :])
                                for bb in range(4):
                                    tqi = ch * 4 + bb
                                    ot = pst.tile([128, 128], F32R, tag="pst")
                                    nc.tensor.transpose(ot[:, 0:72],
                                                        ouT[:, bb * 128:(bb + 1) * 128],
                                                        ident_r[0:72, 0:72])
                                    r = asm.tile([128, 1], F32, tag="recip")
                                    nc.vector.reciprocal(
                                        out=r[:], in_=ot[:, DH:DH + 1].bitcast(F32))
                                    nc.vector.tensor_scalar_mul(
                                        out=opair[:, tqi, base:base + DH],
                                        in0=ot[:, 0:DH].bitcast(F32), scalar1=r[:])
                        for t in range(TQ // 128):
                            po = pst.tile([128, 128], F32, tag="pst")
                            nc.tensor.transpose(po[:], opair[:, t, :], ident_f[:])
                            nc.scalar.copy(out=oT[:, pair, t * 128:(t + 1) * 128],
                                           in_=po[:])

                # ------------- Stage D: Wo + residual + LN2 -------------
                aoT = arena.tile([128, NKO, TQ], F32, tag="arena")
                xn2T = fm4.tile([128, NKO, TQ], F32R, tag="fm4")
                with tc.tile_pool(name="dres", bufs=1) as dres, \
                     tc.tile_pool(name="workD", bufs=2) as workD:
                    wo_s = dres.tile([128, NKO, C], F32R, tag="wo")
                    nc.sync.dma_start(out=wo_s[:], in_=wo_d.rearrange("(o p) f -> p o f", p=128))
                    ln2g_r = dres.tile([128, C], F32, tag="g2")
                    ln2b_r = dres.tile([128, C], F32, tag="b2")
                    nc.gpsimd.dma_start(out=ln2g_r[:], in_=bcast(ln2g_d[:]))
                    nc.gpsimd.dma_start(out=ln2b_r[:], in_=bcast(ln2b_d[:]))

                    for f in range(NKO):
                        for ch in range(TQ // 512):
                            pa = ps.tile([128, 512], F32, tag="ps")
                            for ko in range(NKO):
                                nc.tensor.matmul(pa[:], wo_s[:, ko, f * 128:(f + 1) * 128],
                                                 oT[:, ko, ch * 512:(ch + 1) * 512],
                                                 start=(ko == 0), stop=(ko == NKO - 1))
                            nc.scalar.activation(out=aoT[:, f, ch * 512:(ch + 1) * 512],
                                                 in_=pa[:],
                                                 func=mybir.ActivationFunctionType.Identity,
                                                 bias=bo_s[:, f:f + 1], scale=1.0)
                    for t in range(TQ // 128):
                        x_t = workD.tile([128, C], F32, tag="x_t")
                        nc.sync.dma_start(out=x_t[:], in_=x_d[t * 128:(t + 1) * 128, :])
                        x2_t = workD.tile([128, C], F32, tag="x2_t")
                        for c in range(NKO):
                            pt = pst.tile([128, 128], F32, tag="pst")
                            nc.tensor.transpose(pt[:], aoT[:, c, t * 128:(t + 1) * 128],
                                                ident_f[:])
                            nc.vector.tensor_add(out=x2_t[:, c * 128:(c + 1) * 128],
                                                 in0=pt[:], in1=x_t[:, c * 128:(c + 1) * 128])
                        nc.sync.dma_start(out=x2_h[t], in_=x2_t[:])
                        xn2_r = workD.tile([128, C], F32R, tag="xn_r")
                        _layernorm_tile(nc, stats, workD, x2_t[:], ln2g_r, ln2b_r, eps_t,
                                        xn2_r[:])
                        for c in range(NKO):
                            pt = pst.tile([128, 128], F32R, tag="pst")
                            nc.tensor.transpose(pt[:], xn2_r[:, c * 128:(c + 1) * 128],
                                                ident_r[:])
                            nc.scalar.copy(out=xn2T[:, c, t * 128:(t + 1) * 128], in_=pt[:])

                # ------------- Stage E: FFN up (W1, relu) -------------
                h1T = arena.tile([128, DFF // 128, TQ], BF16, tag="arena")
                with tc.tile_pool(name="w1p", bufs=2) as w1p:
                    for blk in range(DFF // 512):
                        w1t = w1p.tile([128, NKO, 512], F32R, tag="w1t")
                        nc.sync.dma_start(
                            out=w1t[:],
                            in_=w1_d.rearrange("(o p) f -> p o f", p=128)[:, :, blk * 512:(blk + 1) * 512])
                        for fs in range(4):
                            f = blk * 4 + fs
                            for ch in range(TQ // 512):
                                ph = ps.tile([128, 512], F32, tag="ps")
                                for ko in range(NKO):
                                    nc.tensor.matmul(ph[:], w1t[:, ko, fs * 128:(fs + 1) * 128],
                                                     xn2T[:, ko, ch * 512:(ch + 1) * 512],
                                                     start=(ko == 0), stop=(ko == NKO - 1))
                                nc.scalar.activation(out=h1T[:, f, ch * 512:(ch + 1) * 512],
                                                     in_=ph[:],
                                                     func=mybir.ActivationFunctionType.Relu,
                                                     bias=b1_s[:, f:f + 1], scale=1.0)

                # ------------- Stage F: FFN down (W2) + residual + out -------------
                ffnT = fm4.tile([128, NKO, TQ], F32, tag="fm4")
                with tc.tile_pool(name="w2p", bufs=2) as w2p:
                    for f in range(NKO):
                        w2t = w2p.tile([128, DFF // 128, 128], BF16, tag="w2t")
                        nc.sync.dma_start(
                            out=w2t[:],
                            in_=w2_d.rearrange("(o p) f -> p o f", p=128)[:, :, f * 128:(f + 1) * 128])
                        for ch in range(TQ // 512):
                            po2 = ps.tile([128, 512], F32, tag="ps")
                            for ko in range(DFF // 128):
                                nc.tensor.matmul(po2[:], w2t[:, ko, :],
                                                 h1T[:, ko, ch * 512:(ch + 1) * 512],
                                                 start=(ko == 0), stop=(ko == DFF // 128 - 1))
                            nc.scalar.activation(out=ffnT[:, f, ch * 512:(ch + 1) * 512],
                                                 in_=po2[:],
                                                 func=mybir.ActivationFunctionType.Identity,
                                                 bias=b2_s[:, f:f + 1], scale=1.0)
                with tc.tile_pool(name="workF", bufs=2) as workF:
                    for t in range(TQ // 128):
                        x2_t = workF.tile([128, C], F32, tag="x2_t")
                        nc.sync.dma_start(out=x2_t[:], in_=x2_h[t])
                        out_t = workF.tile([128, C], F32, tag="out_t")
                        for c in range(NKO):
                            pt = pst.tile([128, 128], F32, tag="pst")
                            nc.tensor.transpose(pt[:], ffnT[:, c, t * 128:(t + 1) * 128],
                                                ident_f[:])
                            nc.vector.tensor_add(out=out_t[:, c * 128:(c + 1) * 128],
                                                 in0=pt[:], in1=x2_t[:, c * 128:(c + 1) * 128])
                        nc.sync.dma_start(out=out_d[t * 128:(t + 1) * 128, :], in_=out_t[:])

    nc.finalize()
    _legalize_sem_waits(nc)
    return nc


_NC_CACHE = None


def _get_nc():
    global _NC_CACHE
    if _NC_CACHE is None:
        _NC_CACHE = _build_nc()
    return _NC_CACHE


def _shard_inputs(inputs):
    x = np.asarray(inputs["x"], np.float32)
    wq = np.ascontiguousarray(np.transpose(np.asarray(inputs["Wq"], np.float32), (1, 0, 2)).reshape(C, C))
    wk = np.ascontiguousarray(np.transpose(np.asarray(inputs["Wk"], np.float32), (1, 0, 2)).reshape(C, C))
    wv = np.ascontiguousarray(np.transpose(np.asarray(inputs["Wv"], np.float32), (1, 0, 2)).reshape(C, C))
    wo = np.ascontiguousarray(np.asarray(inputs["Wo"], np.float32))
    w1 = np.ascontiguousarray(np.asarray(inputs["W1"], np.float32))
    w2 = np.asarray(inputs["W2"], np.float32).astype(ml_dtypes.bfloat16)
    shared = {
        "wq": wq, "wk": wk, "wv": wv, "wo": wo, "w1": w1, "w2": w2,
        "bq": np.asarray(inputs["bq"], np.float32).reshape(C),
        "bk": np.asarray(inputs["bk"], np.float32).reshape(C),
        "bv": np.asarray(inputs["bv"], np.float32).reshape(C),
        "bo": np.asarray(inputs["bo"], np.float32).reshape(C),
        "b1": np.asarray(inputs["b1"], np.float32).reshape(DFF),
        "b2": np.asarray(inputs["b2"], np.float32).reshape(C),
        "ln1g": np.asarray(inputs["ln1_g"], np.float32),
        "ln1b": np.asarray(inputs["ln1_b"], np.float32),
        "ln2g": np.asarray(inputs["ln2_g"], np.float32),
        "ln2b": np.asarray(inputs["ln2_b"], np.float32),
    }
    in_maps = []
    for c in range(N_CORES):
        b, half = c // 2, c % 2
        own = x[b, half * TQ:(half + 1) * TQ]
        other = x[b, (1 - half) * TQ:(2 - half) * TQ]
        x_perm = np.ascontiguousarray(np.concatenate([own, other], axis=0))
        in_maps.append(dict(shared, x=x_perm))
    return in_maps


def _run(inputs, **spmd_kwargs):
    nc = _get_nc()
    in_maps = _shard_inputs(inputs)
    res = run_bass_kernel_spmd(nc, in_maps, core_ids=list(range(N_CORES)), **spmd_kwargs)
    out = np.empty((B, T, C), np.float32)
    for c in range(N_CORES):
        b, half = c // 2, c % 2
        out[b, half * TQ:(half + 1) * TQ] = res.results[c]["out"]
    return out, res


def kernel(**inputs) -> np.ndarray:
    out, _ = _run(inputs)
    return out
